# revision 2
# baseline (speedup 1.0000x reference)
"""Multi-task MoE routing (nn_CGC_69836168233304) on 8 TRN2 NeuronCores.

HW exec ~684us (prev baseline 823us). PE-bound at the fp16 matmul-stream
ceiling: measured per-MM N=512 fp16 ~252ns / fp8-DR ~208ns on this stack,
so 384 expert groups x ~1720ns + gate ~= 675us of irreducible PE time for
the KF=6 precision split (error budget blocks more fp8: KF=5 sims at
2.4e-2 > 2e-2).

Reference math:
  h[g,e] = relu(x @ W[g,e] + b[g,e])                   12 experts (3 groups x 4)
  sel_t  = softmax(x @ Wg[t] + bg[t])   over 8 cols    t in {0,1}
  sel_s  = softmax(x @ Wgs + bgs)       over 12 cols
  out_t  = sum_m sel_t[:,m] * concat(h[t], h[2])[m]    t in {0,1}
  out_s  = sum_m sel_s[:,m] * concat(h[0],h[1],h[2])[m]

Sharding: data-parallel over batch B=16384 -> 2048 rows/core; every core holds
all 12 expert weights (streamed from HBM per O-slice); host concatenates
shards (no collectives).

Key structure (per core, batch-major):
  - K=1024 split: chunks 0-5 fp16, chunks 6-7 fp8e4m3 DoubleRow (weights
    pre-scaled by 64; 1/64 rides the sel scale). The DR chunk runs FIRST
    in each accumulation group (start=True) so its non-FWL LDWEIGHTS
    overlaps the previous group's fp16 stream.
  - experts processed in PAIRS sharing one [128,2,512] 2-bank PSUM tile;
    ONE DVE tensor_tensor adds the bias for both experts and drains both
    banks (amortizes the ~120-cycle PSUM-read init over 1024 lanes). No
    PE bias matmuls at all.
  - bias replication across partitions via 0-stride (partition_broadcast)
    DMAs, split per expert-pair and issued for both O-slices up front so
    the first bias TT of each slice never waits on a monolithic 3MB DMA.
  - products: 2-contribution experts via ACT relu(scale*tmp2) passes
    (ACT has slack); 3-contribution experts via 4x-mode 2-op
    tensor_scalar (max0, mult) on DVE, except one product per expert
    routed to ACT to balance queues. scalar_tensor_tensor was tried for
    fused product+add but runs 1x-mode (594ns) -- worse than TS+TT.
  - accumulate fp16 tensor_tensor adds; 3 first-touches write acc directly.
  - out DMA per (bt, osl) from inside the LAST pair's bt loop.

Engine model per (osl,bt) tile-set (measured rates): PE 21.0us,
DVE 18.1us, ACT 13.3us -> PE is the wall; measured wall is PE + ~10us
coupling. Rebalancing further onto PE (e.g. ones-matmul biases) or DVE
(all-TS products) measures slower.
"""

import numpy as np

import concourse.bacc as bacc
import concourse.mybir as mybir
import concourse.tile as tile
from concourse.bass_utils import run_bass_kernel_spmd

F32 = mybir.dt.float32
F16 = mybir.dt.float16
F8 = mybir.dt.float8e4

N_CORES = 8
B, D, O = 16384, 1024, 1024
BC = B // N_CORES
NE = 12  # experts
NG = 28  # gate columns: 8 (task0) + 8 (task1) + 12 (shared)
SEGS = ((0, 8), (8, 16), (16, 28))
OSL = 512  # output-column slice per PSUM bank
KB = 8  # 128-row K chunks in D
KF = 6  # chunks 0..KF-1 in fp16; chunks KF..7 in fp8 DoubleRow
WSCALE = 64.0  # weight pre-scale (fp16 and fp8), undone in the sel scale
N_BT = BC // 128  # 16 batch tiles per core

# pairs of experts sharing one 2-bank psum tile; 3-contribution experts
# (8..11) first so the three accumulator planes first-touch at e=8 and the
# gate softmax interleaves with their bias-only drains.
E_PAIRS = [(8, 9), (10, 11), (0, 1), (2, 3), (4, 5), (6, 7)]


def _contribs(e):
    """(out_k, sel_col) pairs for expert e. Gate col order: t0=[g0e0..3,g2e0..3],
    t1=[g1e0..3,g2e0..3], shared=[g0,g1,g2]."""
    if e < 4:
        return [(0, e), (2, 16 + e)]
    if e < 8:
        return [(1, 8 + (e - 4)), (2, 20 + (e - 4))]
    return [(0, 4 + (e - 8)), (1, 12 + (e - 8)), (2, 24 + (e - 8))]


def _build(reps=1):
    N_OSL = O // OSL

    nc = bacc.Bacc("TRN2", target_bir_lowering=False, debug=False)

    # x pre-tiled per batch-tile with 2KB/partition contiguous lines:
    # xbt[bt, p, k, c] = x[bt*128+c, k*128+p]
    xbt_d = nc.dram_tensor("xbt", [N_BT, 128, KB, 128], F16, kind="ExternalInput")
    x8_d = nc.dram_tensor("x8bt", [N_BT, 128, 2, 128], F8, kind="ExternalInput")
    w16_d = nc.dram_tensor("We16", [NE, KF * 128, O], F16, kind="ExternalInput")
    w8_d = nc.dram_tensor("We8", [NE, 128, 2, O], F8, kind="ExternalInput")
    wg_d = nc.dram_tensor("Wgc", [D, NG], F16, kind="ExternalInput")
    bg_d = nc.dram_tensor("bgc", [1, NG], F16, kind="ExternalInput")
    be_d = nc.dram_tensor("Bef", [NE, O], F32, kind="ExternalInput")
    out_d = nc.dram_tensor("outA", [3, BC, O], F16, kind="ExternalOutput")

    with tile.TileContext(nc) as tc:
        with (
            tc.tile_pool(name="big", bufs=1) as big,
            tc.tile_pool(name="wpool", bufs=4) as wpool,
            tc.tile_pool(name="accp", bufs=1) as accp,
            tc.tile_pool(name="tmpp", bufs=8) as tmpp,
            tc.tile_pool(name="gatep", bufs=2) as gatep,
            tc.tile_pool(name="psum", bufs=3, space="PSUM") as psum,
            tc.tile_pool(name="psumg", bufs=1, space="PSUM") as psumg,
        ):
            for rep in range(reps):
                # --- small resident staging ---
                wg_sb = big.tile([128, KB, NG], F16, tag="wg")
                nc.sync.dma_start(
                    wg_sb[:], wg_d.ap().rearrange("(k p) g -> p k g", p=128)
                )
                bg_sb = big.tile([1, NG], F16, tag="bg")
                nc.sync.dma_start(bg_sb[:], bg_d.ap())
                ones_sb = big.tile([1, 128], F16, tag="ones")
                nc.vector.memset(ones_sb[:], 1.0)

                # x stream: first bt slice now, the rest prefetched one-ahead
                # from inside the first pair's bt loop.
                x_sb = big.tile([128, N_BT, KB, 128], F16, tag="x")
                x8_sb = big.tile([128, N_BT, 2, 128], F8, tag="x8")
                nc.sync.dma_start(x_sb[:, 0], xbt_d.ap()[0])
                nc.sync.dma_start(x8_sb[:, 0], x8_d.ap()[0])

                # sel64[:, bt, col] = softmax(...)[col] / WSCALE
                sel_sb = big.tile([128, N_BT, NG], F32, tag="sel")

                def emit_gate(bt):
                    pg = psumg.tile([128, NG], F32)
                    for k in range(KB):
                        nc.tensor.matmul(
                            pg[:], x_sb[:, bt, k, :], wg_sb[:, k, :],
                            start=(k == 0), stop=False,
                        )
                    nc.tensor.matmul(
                        pg[:], ones_sb[:], bg_sb[:], start=False, stop=True
                    )
                    et = gatep.tile([128, NG], F32)
                    nc.scalar.activation(
                        et[:], pg[:], mybir.ActivationFunctionType.Exp
                    )
                    for s0, s1 in SEGS:
                        den = gatep.tile([128, 1], F32, tag="den")
                        nc.vector.tensor_reduce(
                            den[:], et[:, s0:s1], mybir.AxisListType.X,
                            mybir.AluOpType.add,
                        )
                        den64 = gatep.tile([128, 1], F32, tag="den64")
                        nc.vector.tensor_scalar(
                            den64[:], den[:], WSCALE, None, mybir.AluOpType.mult
                        )
                        rden = gatep.tile([128, 1], F32, tag="rden")
                        nc.vector.reciprocal(rden[:], den64[:])
                        nc.vector.tensor_scalar(
                            sel_sb[:, bt, s0:s1], et[:, s0:s1], rden[:], None,
                            mybir.AluOpType.mult,
                        )

                # --- experts + gated accumulation ---
                # 64*b replicated across partitions via 0-stride DMAs,
                # split per expert-pair so the first bias TT only waits on
                # its own 512KB slice (not one monolithic 3MB broadcast)
                breps = {}
                for osl in range(N_OSL):
                    breps[osl] = big.tile(
                        [128, NE, OSL], F32, tag=f"brep{osl}", name="brep"
                    )
                for osl in range(N_OSL):
                    o0 = osl * OSL
                    for pair in E_PAIRS:
                        i0 = pair[0]
                        nc.sync.dma_start(
                            breps[osl][:, i0 : i0 + 2, :],
                            be_d.ap()[
                                i0 : i0 + 2, o0 : o0 + OSL
                            ].partition_broadcast(128),
                        )
                for osl in range(N_OSL):
                    o0 = osl * OSL
                    brep = breps[osl]
                    touched = set()
                    acct = {}
                    for pi, pair in enumerate(E_PAIRS):
                        first_p = pi == 0
                        last_p = pi == len(E_PAIRS) - 1
                        w_sbs = []
                        w8_sbs = []
                        for e in pair:
                            w_sb = wpool.tile([128, KF, OSL], F16, tag="w16")
                            nc.sync.dma_start(
                                w_sb[:],
                                w16_d.ap()[e, :, o0 : o0 + OSL].rearrange(
                                    "(k p) o -> p k o", p=128
                                ),
                            )
                            w8_sb = wpool.tile([128, 2, OSL], F8, tag="w8")
                            nc.sync.dma_start(
                                w8_sb[:], w8_d.ap()[e, :, :, o0 : o0 + OSL]
                            )
                            w_sbs.append(w_sb)
                            w8_sbs.append(w8_sb)
                        for bt in range(N_BT):
                            if first_p and osl == 0:
                                if bt + 1 < N_BT:  # prefetch next x slice
                                    nc.sync.dma_start(
                                        x_sb[:, bt + 1], xbt_d.ap()[bt + 1]
                                    )
                                    nc.sync.dma_start(
                                        x8_sb[:, bt + 1], x8_d.ap()[bt + 1]
                                    )
                                emit_gate(bt)
                            ps = psum.tile([128, 2, OSL], F32, name="ps")
                            for j in range(2):
                                nc.tensor.matmul(
                                    ps[:, j, :],
                                    x8_sb[:, bt],
                                    w8_sbs[j][:],
                                    start=True,
                                    stop=False,
                                    perf_mode=mybir.MatmulPerfMode.DoubleRow,
                                )
                                for k in range(KF):
                                    nc.tensor.matmul(
                                        ps[:, j, :],
                                        x_sb[:, bt, k, :],
                                        w_sbs[j][:, k, :],
                                        start=False,
                                        stop=(k == KF - 1),
                                    )
                            # paired bias-add drains both banks in one TT:
                            # tmp2 = ps + 64b (fp16)
                            i0 = pair[0]
                            tmp2 = tmpp.tile([128, 2, OSL], F16, tag="tmp2")
                            nc.vector.tensor_tensor(
                                tmp2[:], ps[:], brep[:, i0 : i0 + 2, :],
                                mybir.AluOpType.add,
                            )
                            if bt not in acct:
                                a = accp.tile(
                                    [128, 3, OSL], F16, tag=f"acc{bt}",
                                    name=f"acc{bt}",
                                )
                                acct[bt] = a
                            a = acct[bt]
                            prods = []  # (k, product tile) pending adds
                            for j, e in enumerate(pair):
                                src = tmp2[:, j, :]
                                for ci, (k, col) in enumerate(_contribs(e)):
                                    sc = sel_sb[:, bt, col : col + 1]
                                    if (k, bt) not in touched:
                                        touched.add((k, bt))
                                        dst = a[:, k, :]
                                        direct = True
                                    else:
                                        p = tmpp.tile(
                                            [128, OSL], F16, tag="prod"
                                        )
                                        prods.append((k, p))
                                        dst = p[:]
                                        direct = False
                                    if e < 8 or (ci == 2 and not direct):
                                        # product via ACT: relu(tmp2*sel64)
                                        nc.scalar.activation(
                                            dst, src,
                                            mybir.ActivationFunctionType.Relu,
                                            scale=sc,
                                        )
                                    else:
                                        # product via 4x-mode 2-op
                                        # tensor_scalar: (tmp2 max 0)*sel64
                                        nc.vector.tensor_scalar(
                                            dst, src, 0.0, sc,
                                            mybir.AluOpType.max,
                                            mybir.AluOpType.mult,
                                        )
                            for k, p in prods:
                                nc.vector.tensor_tensor(
                                    a[:, k, :], a[:, k, :], p[:],
                                    mybir.AluOpType.add,
                                )
                            if last_p:
                                nc.sync.dma_start(
                                    out_d.ap()[
                                        :, bt * 128 : (bt + 1) * 128,
                                        o0 : o0 + OSL,
                                    ].rearrange("k p o -> p k o"),
                                    a[:],
                                )

    nc.compile()
    return nc


_NC_CACHE = None


def get_nc():
    global _NC_CACHE
    if _NC_CACHE is None:
        _NC_CACHE = _build()
    return _NC_CACHE


def build_timing(reps):
    return _build(reps=reps)


def make_in_maps(inputs):
    x = np.asarray(inputs["x"], dtype=np.float32)
    W = np.asarray(inputs["W"], dtype=np.float32).reshape(NE, D, O)
    b = np.asarray(inputs["b"], dtype=np.float32).reshape(NE, O)
    Wg = np.asarray(inputs["Wg"], dtype=np.float32)
    bg = np.asarray(inputs["bg"], dtype=np.float32)
    Wgs = np.asarray(inputs["Wgs"], dtype=np.float32)
    bgs = np.asarray(inputs["bgs"], dtype=np.float32)

    import ml_dtypes

    f8 = ml_dtypes.float8_e4m3

    # fp16 chunks 0..KF-1, pre-scaled by WSCALE so PSUM is uniformly 64x
    w16 = np.ascontiguousarray(
        (W[:, : KF * 128, :] * WSCALE).astype(np.float16)
    )
    # fp8 chunks KF..7: [NE, 128, 2, O], 64*W
    w8 = np.ascontiguousarray(
        (W[:, KF * 128 :, :] * WSCALE)
        .reshape(NE, 2, 128, O)
        .transpose(0, 2, 1, 3)
    ).astype(f8)

    shared = {
        "We16": w16,
        "We8": w8,
        "Bef": (b * WSCALE).astype(np.float32),
        "Wgc": np.concatenate([Wg[0], Wg[1], Wgs], axis=1).astype(np.float16),
        "bgc": np.concatenate([bg[0], bg[1], bgs])[None, :].astype(np.float16),
    }
    in_maps = []
    for c in range(N_CORES):
        m = dict(shared)
        xc = x[c * BC : (c + 1) * BC]
        # xbt[bt, p, k, c] = x[bt*128+c, k*128+p]
        m["xbt"] = np.ascontiguousarray(
            xc.reshape(N_BT, 128, KB, 128).transpose(0, 3, 2, 1)
        ).astype(np.float16)
        m["x8bt"] = np.ascontiguousarray(
            xc[:, KF * 128 :]
            .reshape(N_BT, 128, 2, 128)
            .transpose(0, 3, 2, 1)
        ).astype(f8)
        in_maps.append(m)
    return in_maps


def kernel(x, W, b, Wg, bg, Wgs, bgs):
    nc = get_nc()
    in_maps = make_in_maps(
        {"x": x, "W": W, "b": b, "Wg": Wg, "bg": bg, "Wgs": Wgs, "bgs": bgs}
    )
    res = run_bass_kernel_spmd(nc, in_maps, list(range(N_CORES)))
    return tuple(
        np.concatenate(
            [res.results[c]["outA"][k] for c in range(N_CORES)], axis=0
        ).astype(np.float32)
        for k in range(3)
    )


# revision 3
# speedup vs baseline: 34.8219x; 34.8219x over previous
"""Multi-task MoE routing (nn_CGC_69836168233304) on 8 TRN2 NeuronCores.

HW exec ~690-745us depending on device state (prev baseline 823us;
drift-controlled A/B vs the non-staggered variant: 732 vs 745us median). PE-bound at the fp16 matmul-stream
ceiling: measured per-MM N=512 fp16 ~252ns / fp8-DR ~208ns on this stack,
so 384 expert groups x ~1720ns + gate ~= 675us of irreducible PE time for
the KF=6 precision split (error budget blocks more fp8: KF=5 sims at
2.4e-2 > 2e-2).

Reference math:
  h[g,e] = relu(x @ W[g,e] + b[g,e])                   12 experts (3 groups x 4)
  sel_t  = softmax(x @ Wg[t] + bg[t])   over 8 cols    t in {0,1}
  sel_s  = softmax(x @ Wgs + bgs)       over 12 cols
  out_t  = sum_m sel_t[:,m] * concat(h[t], h[2])[m]    t in {0,1}
  out_s  = sum_m sel_s[:,m] * concat(h[0],h[1],h[2])[m]

Sharding: data-parallel over batch B=16384 -> 2048 rows/core; every core holds
all 12 expert weights (streamed from HBM per O-slice); host concatenates
shards (no collectives).

Key structure (per core, batch-major):
  - K=1024 split: chunks 0-5 fp16, chunks 6-7 fp8e4m3 DoubleRow (weights
    pre-scaled by 64; 1/64 rides the sel scale). The DR chunk runs LAST in
    each accumulation group (DR-first measured slower: 1877 vs 1733
    ns/group in isolation).
  - experts processed in PAIRS sharing one [128,2,512] 2-bank PSUM tile;
    ONE DVE tensor_tensor adds the bias for both experts and drains both
    banks (amortizes the ~120-cycle PSUM-read init over 1024 lanes). No
    PE bias matmuls at all.
  - bias replication across partitions via 0-stride (partition_broadcast)
    DMAs, split per expert-pair and STAGGERED from inside the pair loop
    (only pair 0's slice up front) so ~6MB of broadcast traffic never
    queues ahead of the weight DMAs the PE waits on at rep start.
  - products: 2-contribution experts via ACT relu(scale*tmp2) passes
    (ACT has slack); 3-contribution experts via 4x-mode 2-op
    tensor_scalar (max0, mult) on DVE, except one product per expert
    routed to ACT to balance queues. scalar_tensor_tensor was tried for
    fused product+add but runs 1x-mode (594ns) -- worse than TS+TT.
  - accumulate fp16 tensor_tensor adds; 3 first-touches write acc directly.
  - out DMA per (bt, osl) from inside the LAST pair's bt loop.

Engine model per (osl,bt) tile-set (measured rates): PE 21.0us,
DVE 18.1us, ACT 13.3us -> PE is the wall; measured wall is PE + ~10us
coupling. Rebalancing further onto PE (e.g. ones-matmul biases) or DVE
(all-TS products) measures slower.
"""

import numpy as np

import concourse.bacc as bacc
import concourse.mybir as mybir
import concourse.tile as tile
from concourse.bass_utils import run_bass_kernel_spmd

F32 = mybir.dt.float32
F16 = mybir.dt.float16
F8 = mybir.dt.float8e4

N_CORES = 8
B, D, O = 16384, 1024, 1024
BC = B // N_CORES
NE = 12  # experts
NG = 28  # gate columns: 8 (task0) + 8 (task1) + 12 (shared)
SEGS = ((0, 8), (8, 16), (16, 28))
OSL = 512  # output-column slice per PSUM bank
KB = 8  # 128-row K chunks in D
KF = 6  # chunks 0..KF-1 in fp16; chunks KF..7 in fp8 DoubleRow
WSCALE = 64.0  # weight pre-scale (fp16 and fp8), undone in the sel scale
N_BT = BC // 128  # 16 batch tiles per core

# pairs of experts sharing one 2-bank psum tile; 3-contribution experts
# (8..11) first so the three accumulator planes first-touch at e=8 and the
# gate softmax interleaves with their bias-only drains.
E_PAIRS = [(8, 9), (10, 11), (0, 1), (2, 3), (4, 5), (6, 7)]


def _contribs(e):
    """(out_k, sel_col) pairs for expert e. Gate col order: t0=[g0e0..3,g2e0..3],
    t1=[g1e0..3,g2e0..3], shared=[g0,g1,g2]."""
    if e < 4:
        return [(0, e), (2, 16 + e)]
    if e < 8:
        return [(1, 8 + (e - 4)), (2, 20 + (e - 4))]
    return [(0, 4 + (e - 8)), (1, 12 + (e - 8)), (2, 24 + (e - 8))]


def _build(reps=1):
    N_OSL = O // OSL

    nc = bacc.Bacc("TRN2", target_bir_lowering=False, debug=False)

    # x pre-tiled per batch-tile with 2KB/partition contiguous lines:
    # xbt[bt, p, k, c] = x[bt*128+c, k*128+p]
    xbt_d = nc.dram_tensor("xbt", [N_BT, 128, KB, 128], F16, kind="ExternalInput")
    x8_d = nc.dram_tensor("x8bt", [N_BT, 128, 2, 128], F8, kind="ExternalInput")
    w16_d = nc.dram_tensor("We16", [NE, KF * 128, O], F16, kind="ExternalInput")
    w8_d = nc.dram_tensor("We8", [NE, 128, 2, O], F8, kind="ExternalInput")
    wg_d = nc.dram_tensor("Wgc", [D, NG], F16, kind="ExternalInput")
    bg_d = nc.dram_tensor("bgc", [1, NG], F16, kind="ExternalInput")
    be_d = nc.dram_tensor("Bef", [NE, O], F32, kind="ExternalInput")
    out_d = nc.dram_tensor("outA", [3, BC, O], F16, kind="ExternalOutput")

    with tile.TileContext(nc) as tc:
        with (
            tc.tile_pool(name="big", bufs=1) as big,
            tc.tile_pool(name="wpool", bufs=6) as wpool,
            tc.tile_pool(name="accp", bufs=1) as accp,
            tc.tile_pool(name="tmpp", bufs=8) as tmpp,
            tc.tile_pool(name="gatep", bufs=2) as gatep,
            tc.tile_pool(name="psum", bufs=3, space="PSUM") as psum,
            tc.tile_pool(name="psumg", bufs=1, space="PSUM") as psumg,
        ):
            for rep in range(reps):
                # --- small resident staging ---
                wg_sb = big.tile([128, KB, NG], F16, tag="wg")
                nc.sync.dma_start(
                    wg_sb[:], wg_d.ap().rearrange("(k p) g -> p k g", p=128)
                )
                bg_sb = big.tile([1, NG], F16, tag="bg")
                nc.sync.dma_start(bg_sb[:], bg_d.ap())
                ones_sb = big.tile([1, 128], F16, tag="ones")
                nc.vector.memset(ones_sb[:], 1.0)

                # x stream: first bt slice now, the rest prefetched one-ahead
                # from inside the first pair's bt loop.
                x_sb = big.tile([128, N_BT, KB, 128], F16, tag="x")
                x8_sb = big.tile([128, N_BT, 2, 128], F8, tag="x8")
                nc.sync.dma_start(x_sb[:, 0], xbt_d.ap()[0])
                nc.sync.dma_start(x8_sb[:, 0], x8_d.ap()[0])

                # sel64[:, bt, col] = softmax(...)[col] / WSCALE
                sel_sb = big.tile([128, N_BT, NG], F32, tag="sel")

                def emit_gate(bt):
                    pg = psumg.tile([128, NG], F32)
                    for k in range(KB):
                        nc.tensor.matmul(
                            pg[:], x_sb[:, bt, k, :], wg_sb[:, k, :],
                            start=(k == 0), stop=False,
                        )
                    nc.tensor.matmul(
                        pg[:], ones_sb[:], bg_sb[:], start=False, stop=True
                    )
                    et = gatep.tile([128, NG], F32)
                    nc.scalar.activation(
                        et[:], pg[:], mybir.ActivationFunctionType.Exp
                    )
                    for s0, s1 in SEGS:
                        den = gatep.tile([128, 1], F32, tag="den")
                        nc.vector.tensor_reduce(
                            den[:], et[:, s0:s1], mybir.AxisListType.X,
                            mybir.AluOpType.add,
                        )
                        den64 = gatep.tile([128, 1], F32, tag="den64")
                        nc.vector.tensor_scalar(
                            den64[:], den[:], WSCALE, None, mybir.AluOpType.mult
                        )
                        rden = gatep.tile([128, 1], F32, tag="rden")
                        nc.vector.reciprocal(rden[:], den64[:])
                        nc.vector.tensor_scalar(
                            sel_sb[:, bt, s0:s1], et[:, s0:s1], rden[:], None,
                            mybir.AluOpType.mult,
                        )

                # --- experts + gated accumulation ---
                # 64*b replicated across partitions via 0-stride DMAs,
                # split per expert-pair. Only the first pair's slice is
                # issued up front; the rest are staggered from inside the
                # pair loop so 6MB of broadcast traffic never queues ahead
                # of the first weight DMAs (which the PE waits on).
                breps = {}
                for osl in range(N_OSL):
                    breps[osl] = big.tile(
                        [128, NE, OSL], F32, tag=f"brep{osl}", name="brep"
                    )

                brep_emitted = set()

                def emit_brep(osl, pi):
                    if (osl, pi) in brep_emitted:
                        return
                    brep_emitted.add((osl, pi))
                    o0 = osl * OSL
                    i0 = E_PAIRS[pi][0]
                    nc.sync.dma_start(
                        breps[osl][:, i0 : i0 + 2, :],
                        be_d.ap()[
                            i0 : i0 + 2, o0 : o0 + OSL
                        ].partition_broadcast(128),
                    )

                emit_brep(0, 0)
                for osl in range(N_OSL):
                    o0 = osl * OSL
                    brep = breps[osl]
                    touched = set()
                    acct = {}
                    for pi, pair in enumerate(E_PAIRS):
                        first_p = pi == 0
                        last_p = pi == len(E_PAIRS) - 1
                        w_sbs = []
                        w8_sbs = []
                        for e in pair:
                            w_sb = wpool.tile([128, KF, OSL], F16, tag="w16")
                            nc.sync.dma_start(
                                w_sb[:],
                                w16_d.ap()[e, :, o0 : o0 + OSL].rearrange(
                                    "(k p) o -> p k o", p=128
                                ),
                            )
                            w8_sb = wpool.tile([128, 2, OSL], F8, tag="w8")
                            nc.sync.dma_start(
                                w8_sb[:], w8_d.ap()[e, :, :, o0 : o0 + OSL]
                            )
                            w_sbs.append(w_sb)
                            w8_sbs.append(w8_sb)
                        # stagger the bias-broadcast DMAs behind the weights:
                        # next pair's slice this osl, and this pair's slice
                        # for the following osl
                        if pi + 1 < len(E_PAIRS):
                            emit_brep(osl, pi + 1)
                        if osl + 1 < N_OSL:
                            emit_brep(osl + 1, pi)
                        for bt in range(N_BT):
                            if first_p and osl == 0:
                                if bt + 1 < N_BT:  # prefetch next x slice
                                    nc.sync.dma_start(
                                        x_sb[:, bt + 1], xbt_d.ap()[bt + 1]
                                    )
                                    nc.sync.dma_start(
                                        x8_sb[:, bt + 1], x8_d.ap()[bt + 1]
                                    )
                                emit_gate(bt)
                            ps = psum.tile([128, 2, OSL], F32, name="ps")
                            for j in range(2):
                                for k in range(KF):
                                    nc.tensor.matmul(
                                        ps[:, j, :],
                                        x_sb[:, bt, k, :],
                                        w_sbs[j][:, k, :],
                                        start=(k == 0),
                                        stop=False,
                                    )
                                nc.tensor.matmul(
                                    ps[:, j, :],
                                    x8_sb[:, bt],
                                    w8_sbs[j][:],
                                    start=False,
                                    stop=True,
                                    perf_mode=mybir.MatmulPerfMode.DoubleRow,
                                )
                            # paired bias-add drains both banks in one TT:
                            # tmp2 = ps + 64b (fp16)
                            i0 = pair[0]
                            tmp2 = tmpp.tile([128, 2, OSL], F16, tag="tmp2")
                            nc.vector.tensor_tensor(
                                tmp2[:], ps[:], brep[:, i0 : i0 + 2, :],
                                mybir.AluOpType.add,
                            )
                            if bt not in acct:
                                a = accp.tile(
                                    [128, 3, OSL], F16, tag=f"acc{bt}",
                                    name=f"acc{bt}",
                                )
                                acct[bt] = a
                            a = acct[bt]
                            prods = []  # (k, product tile) pending adds
                            for j, e in enumerate(pair):
                                src = tmp2[:, j, :]
                                for ci, (k, col) in enumerate(_contribs(e)):
                                    sc = sel_sb[:, bt, col : col + 1]
                                    if (k, bt) not in touched:
                                        touched.add((k, bt))
                                        dst = a[:, k, :]
                                        direct = True
                                    else:
                                        p = tmpp.tile(
                                            [128, OSL], F16, tag="prod"
                                        )
                                        prods.append((k, p))
                                        dst = p[:]
                                        direct = False
                                    if e < 8 or (ci == 2 and not direct):
                                        # product via ACT: relu(tmp2*sel64)
                                        nc.scalar.activation(
                                            dst, src,
                                            mybir.ActivationFunctionType.Relu,
                                            scale=sc,
                                        )
                                    else:
                                        # product via 4x-mode 2-op
                                        # tensor_scalar: (tmp2 max 0)*sel64
                                        nc.vector.tensor_scalar(
                                            dst, src, 0.0, sc,
                                            mybir.AluOpType.max,
                                            mybir.AluOpType.mult,
                                        )
                            for k, p in prods:
                                nc.vector.tensor_tensor(
                                    a[:, k, :], a[:, k, :], p[:],
                                    mybir.AluOpType.add,
                                )
                            if last_p:
                                nc.sync.dma_start(
                                    out_d.ap()[
                                        :, bt * 128 : (bt + 1) * 128,
                                        o0 : o0 + OSL,
                                    ].rearrange("k p o -> p k o"),
                                    a[:],
                                )

    nc.compile()
    return nc


_NC_CACHE = None


def get_nc():
    global _NC_CACHE
    if _NC_CACHE is None:
        _NC_CACHE = _build()
    return _NC_CACHE


def build_timing(reps):
    return _build(reps=reps)


def make_in_maps(inputs):
    x = np.asarray(inputs["x"], dtype=np.float32)
    W = np.asarray(inputs["W"], dtype=np.float32).reshape(NE, D, O)
    b = np.asarray(inputs["b"], dtype=np.float32).reshape(NE, O)
    Wg = np.asarray(inputs["Wg"], dtype=np.float32)
    bg = np.asarray(inputs["bg"], dtype=np.float32)
    Wgs = np.asarray(inputs["Wgs"], dtype=np.float32)
    bgs = np.asarray(inputs["bgs"], dtype=np.float32)

    import ml_dtypes

    f8 = ml_dtypes.float8_e4m3

    # fp16 chunks 0..KF-1, pre-scaled by WSCALE so PSUM is uniformly 64x
    w16 = np.ascontiguousarray(
        (W[:, : KF * 128, :] * WSCALE).astype(np.float16)
    )
    # fp8 chunks KF..7: [NE, 128, 2, O], 64*W
    w8 = np.ascontiguousarray(
        (W[:, KF * 128 :, :] * WSCALE)
        .reshape(NE, 2, 128, O)
        .transpose(0, 2, 1, 3)
    ).astype(f8)

    shared = {
        "We16": w16,
        "We8": w8,
        "Bef": (b * WSCALE).astype(np.float32),
        "Wgc": np.concatenate([Wg[0], Wg[1], Wgs], axis=1).astype(np.float16),
        "bgc": np.concatenate([bg[0], bg[1], bgs])[None, :].astype(np.float16),
    }
    in_maps = []
    for c in range(N_CORES):
        m = dict(shared)
        xc = x[c * BC : (c + 1) * BC]
        # xbt[bt, p, k, c] = x[bt*128+c, k*128+p]
        m["xbt"] = np.ascontiguousarray(
            xc.reshape(N_BT, 128, KB, 128).transpose(0, 3, 2, 1)
        ).astype(np.float16)
        m["x8bt"] = np.ascontiguousarray(
            xc[:, KF * 128 :]
            .reshape(N_BT, 128, 2, 128)
            .transpose(0, 3, 2, 1)
        ).astype(f8)
        in_maps.append(m)
    return in_maps


def kernel(x, W, b, Wg, bg, Wgs, bgs):
    nc = get_nc()
    in_maps = make_in_maps(
        {"x": x, "W": W, "b": b, "Wg": Wg, "bg": bg, "Wgs": Wgs, "bgs": bgs}
    )
    res = run_bass_kernel_spmd(nc, in_maps, list(range(N_CORES)))
    return tuple(
        np.concatenate(
            [res.results[c]["outA"][k] for c in range(N_CORES)], axis=0
        ).astype(np.float32)
        for k in range(3)
    )
